# revision 1
# baseline (speedup 1.0000x reference)
"""Trainium2 Bass kernel for nn_CrossAttention (dense_transformer).

Problem (per batch-head, 8 of them == 8 cores):
  K = conv1x1(x1, W1g, b1g)   -> [4096, 64]   (raw .view semantics)
  Q = conv1x1(x1, W2g, b2g)   -> [4096, 64]
  V = conv1x1(x2, W3g, b3g)   -> [4096, 64]
  A = K @ Q^T                  -> [4096, 4096]
  P = softmax(A, global over all 16.7M entries)
  O = P @ V                    -> [4096, 64]
  out_partial = (W4_gslice/S) @ O + b4/4   (summed over the 4 head-cores of a
                                            batch on the host during unshard)

Sharding: data-parallel over batch*head (core p <-> (b=p//4, g=p%4)).
Global softmax per bh row block needs no cross-device reduction.

Key implementation points:
 - softmax is computed as exp(A - 64) / sum(exp(A - 64)): a constant shift is
   mathematically identical to max-subtraction (softmax is shift invariant)
   and overflow-safe here (A values are ~N(0, 9.4^2), |A|max ~ 66 << 64+88).
 - the attention matrix is never materialized to HBM: A^T tiles are computed
   in PSUM, exp'd by the scalar engine into SBUF, and immediately consumed by
   the P@V matmul (flash-attention style, one pass).
 - row sums come for free from a ones-column appended to V.
 - all matmuls keep the contraction dim on partitions; K^T/Q^T layouts are
   produced with PE transposes of [128,64] tiles.
"""

import numpy as np

import concourse.bass as bass
import concourse.mybir as mybir
import concourse.tile as tile
from concourse import bacc
from concourse.bass_utils import run_bass_kernel_spmd
from concourse.masks import make_identity

F32 = mybir.dt.float32
AF = mybir.ActivationFunctionType
AX = mybir.AxisListType

C = 256          # channels
N = 4096         # h*w
D = 64           # head dim (== w) and channels per head block
NJT = N // 128   # 32 j-tiles
NIB = N // 512   # 8 i-blocks
GRP = 3          # kq j-tiles per exp group (3 PSUM banks)
SHIFT = -64.0    # softmax stabilization shift (constant, shift-invariant)

_CACHE = {}


def _build():
    import os
    globals()['os'] = os
    build_stage = int(os.environ.get("KSTAGE", "3"))
    nc = bacc.Bacc("TRN2", target_bir_lowering=False)
    _emit(nc, build_stage)
    nc.finalize()
    return nc


def _emit(nc, build_stage):
    with tile.TileContext(nc) as tc, \
         tc.tile_pool(name="dram", bufs=1, space="DRAM") as dram, \
         tc.tile_pool(name="persist", bufs=1) as persist:
        # ---- I/O ----
        def ein(name, shape):
            return dram.tile(shape, F32, kind="ExternalInput", name=name,
                             uniquify=False)

        x1 = ein("x1", [C, N])
        x2 = ein("x2", [C, N])
        w1t = ein("w1t", [C, D])      # [c, cl] = W1[g-slice].T
        w2t = ein("w2t", [C, D])
        w3t = ein("w3t", [C, D])
        w4gt = ein("w4gt", [D, C])    # [cl, o] = W4[:, g-slice].T
        b1g = ein("b1g", [D, 1])
        b2g = ein("b2g", [D, 1])
        b3g = ein("b3g", [D, 1])
        b4q = ein("b4q", [1, C])      # b4 / 4
        out = dram.tile([C, N], F32, kind="ExternalOutput", name="out",
                        uniquify=False)

        # DRAM scratch (layout round-trips)
        knat_d = dram.tile([D, N], F32)   # [cl, (h w)] == [(cl h), w] flat
        qnat_d = dram.tile([D, N], F32)
        vnat_d = dram.tile([D, N], F32)
        onat_d = dram.tile([D, N], F32)   # [w, (cl h)]

        # ---- persistent SBUF ----
        ktsb = persist.tile([128, N], F32)        # K^T [w, i] duplicated rows
        qtsb = persist.tile([128, N], F32)        # Q^T [w, j] duplicated rows
        v2 = persist.tile([128, NJT * (D + 1)], F32)   # V' tiles [j-loc, 65]
        otsb = persist.tile([D + 1, N], F32)      # O^T (+ rowsums in row 64)
        w4full = persist.tile([D + 1, C], F32)    # [cl(+bias row), o]
        w4s = persist.tile([D + 1, C], F32)       # scaled by 1/S
        b1s = persist.tile([D, 1], F32)
        b2s = persist.tile([D, 1], F32)
        b3s = persist.tile([D, 1], F32)
        shift = persist.tile([128, 1], F32)
        ident = persist.tile([128, 128], F32)
        ones_r = persist.tile([1, D], F32)        # for 1/S broadcast matmul
        ssum = persist.tile([D + 1, NIB], F32)    # per-i-block sums (row 64)
        stot = persist.tile([D + 1, 1], F32)
        sinv = persist.tile([D + 1, 1], F32)
        sinv0 = persist.tile([1, 1], F32)
        sinv_b = persist.tile([D, 1], F32)

        nc.gpsimd.memset(shift[:], SHIFT)
        nc.gpsimd.memset(ones_r[:], 1.0)
        make_identity(nc, ident[:])
        nc.sync.dma_start(b1s[:], b1g[:])
        nc.sync.dma_start(b2s[:], b2g[:])
        nc.sync.dma_start(b3s[:], b3g[:])
        nc.sync.dma_start(w4full[0:D, :], w4gt[:])
        nc.sync.dma_start(w4full[D:D + 1, :], b4q[:])
        # ones column of V' (col 64 of each slot)
        v2v = v2.rearrange("p (j c) -> p j c", c=D + 1)
        nc.gpsimd.memset(v2v[:, :, D:D + 1], 1.0)

        with tc.tile_pool(name="xsb", bufs=1) as xpool, \
             tc.tile_pool(name="stage", bufs=4) as stage, \
             tc.tile_pool(name="sps", bufs=4, space="PSUM") as sps, \
             tc.tile_pool(name="tps", bufs=2, space="PSUM") as tps:
            # ---- load x1/x2 (chunked for DMA/compute overlap) ----
            x1a = xpool.tile([128, N], F32)
            x1b = xpool.tile([128, N], F32)
            x2a = xpool.tile([128, N], F32)
            x2b = xpool.tile([128, N], F32)
            for ch in range(8):
                s = slice(ch * 512, (ch + 1) * 512)
                nc.sync.dma_start(x1a[:, s], x1[0:128, s])
                nc.sync.dma_start(x1b[:, s], x1[128:256, s])
            for ch in range(8):
                s = slice(ch * 512, (ch + 1) * 512)
                nc.sync.dma_start(x2a[:, s], x2[0:128, s])
                nc.sync.dma_start(x2b[:, s], x2[128:256, s])

            # ---- projections: nat = W.T @ x + b  -> DRAM ----
            wsb = {}
            for nm, w in (("k", w1t), ("q", w2t), ("v", w3t)):
                wa = stage.tile([128, D], F32, name=f"w{nm}a", bufs=1)
                wb = stage.tile([128, D], F32, name=f"w{nm}b", bufs=1)
                nc.sync.dma_start(wa[:], w[0:128, :])
                nc.sync.dma_start(wb[:], w[128:256, :])
                wsb[nm] = (wa, wb)

            for nm, xa, xb, bia, dst in (("k", x1a, x1b, b1s, knat_d),
                                         ("q", x1a, x1b, b2s, qnat_d),
                                         ("v", x2a, x2b, b3s, vnat_d)):
                wa, wb = wsb[nm]
                for ch in range(8):
                    s = slice(ch * 512, (ch + 1) * 512)
                    pp = sps.tile([D, 512], F32, name="projps")
                    nc.tensor.matmul(pp[:], wa[:], xa[:, s], start=True,
                                     stop=False)
                    nc.tensor.matmul(pp[:], wb[:], xb[:, s], start=False,
                                     stop=True)
                    st = stage.tile([D, 512], F32, name="projsb")
                    nc.vector.tensor_scalar_add(st[:], pp[:], bia[:])
                    nc.sync.dma_start(dst[:, s], st[:])

            # ---- K^T / Q^T via PE transposes of [128, 64] tiles ----
            for src, dstt in ((knat_d, ktsb), (qnat_d, qtsb)):
                srcv = src.rearrange("a (h w) -> (a h) w", w=D)
                for t in range(NJT):
                    nat = stage.tile([128, D], F32, name="natt")
                    nc.sync.dma_start(nat[:], srcv[t * 128:(t + 1) * 128, :])
                    tp = tps.tile([D, 128], F32, name="tp")
                    nc.tensor.transpose(tp[:], nat[:], ident[:])
                    nc.any.tensor_copy(dstt[0:D, t * 128:(t + 1) * 128], tp[:])
                nc.sync.dma_start(dstt[D:128, :], dstt[0:D, :])

            # ---- V' load ----
            vv = vnat_d.rearrange("a (h w) -> (a h) w", w=D)
            for t in range(NJT):
                nc.sync.dma_start(v2v[:, t, 0:D], vv[t * 128:(t + 1) * 128, :])

        # ---- attention: one pass, ACT-bound ----
        if build_stage < 2:
            return
        groups = []
        jt0 = 0
        while jt0 < NJT:
            groups.append(list(range(jt0, min(jt0 + GRP, NJT))))
            jt0 += GRP

        with tc.tile_pool(name="kqps", bufs=2, space="PSUM") as kqps, \
             tc.tile_pool(name="ovps", bufs=2, space="PSUM") as ovps, \
             tc.tile_pool(name="ptsb", bufs=3) as ptsb:
            for ib in range(NIB):
                isl = slice(ib * 512, (ib + 1) * 512)
                ov = ovps.tile([D + 1, 512], F32, name="ov")
                for grp in groups:
                    ng = len(grp)
                    kq = kqps.tile([128, GRP * 512], F32, name="kq")
                    for gi, jt in enumerate(grp):
                        half = gi % 2
                        p0, p1 = half * D, half * D + D
                        nc.tensor.matmul(
                            kq[:, gi * 512:(gi + 1) * 512],
                            qtsb[p0:p1, jt * 128:(jt + 1) * 128],
                            ktsb[p0:p1, isl], start=True, stop=True,
                            tile_position=(p0, 0))
                    pt = ptsb.tile([128, GRP * 512], F32, name="pt")
                    nc.scalar.activation(pt[:, 0:ng * 512], kq[:, 0:ng * 512],
                                         AF.Exp, bias=shift[:], scale=1.0)
                    for gi, jt in enumerate(grp):
                        nc.tensor.matmul(
                            ov[:], v2v[:, jt, :],
                            pt[:, gi * 512:(gi + 1) * 512],
                            start=(jt == 0), stop=(jt == NJT - 1))
                nc.vector.tensor_copy(otsb[:, isl], ov[:])
                nc.vector.reduce_sum(ssum[D:D + 1, ib:ib + 1],
                                     ov[D:D + 1, :], axis=AX.X)

        # ---- tail: S, scale W4, final conv ----
        if build_stage < 3:
            return
        ktail = int(os.environ.get("KTAIL", "9"))
        import os as _os  # noqa
        with tc.tile_pool(name="tailsb", bufs=4) as tsb, \
             tc.tile_pool(name="tailps", bufs=4, space="PSUM") as tps2:
            nc.vector.reduce_sum(stot[D:D + 1, :], ssum[D:D + 1, :], axis=AX.X)
            nc.vector.reciprocal(sinv[D:D + 1, :], stot[D:D + 1, :])
            if ktail < 2:
                return
            nc.sync.dma_start(sinv0[:], sinv[D:D + 1, :])
            if ktail < 3:
                return
            pb = tps2.tile([D, 1], F32, name="pb")
            nc.tensor.matmul(pb[:], ones_r[:], sinv0[:], start=True, stop=True)
            nc.vector.tensor_copy(sinv_b[:], pb[:])
            nc.vector.tensor_scalar_mul(w4s[0:D, :], w4full[0:D, :],
                                        sinv_b[:])
            nc.vector.tensor_copy(w4s[D:D + 1, :], w4full[D:D + 1, :])

            if ktail < 4:
                return
            nc.sync.dma_start(onat_d[:], otsb[0:D, :])
            oconv = tsb.tile([D + 1, N], F32, bufs=1)
            nc.gpsimd.memset(oconv[D:D + 1, :], 1.0)
            # onat [w, (cl h)] -> oconv [cl, (w h)]
            if ktail < 5:
                return
            ov2 = onat_d.rearrange("w (cl h) -> cl w h", cl=D)
            oc3 = oconv[0:D, :].rearrange("cl (w h) -> cl w h", w=D)
            nc.sync.dma_start(oc3, ov2)

            if ktail < 6:
                return
            for oc in range(2):
                for ch in range(8):
                    s = slice(ch * 512, (ch + 1) * 512)
                    pp = tps2.tile([128, 512], F32, name="cvps")
                    nc.tensor.matmul(pp[:], w4s[:, oc * 128:(oc + 1) * 128],
                                     oconv[:, s], start=True, stop=True)
                    ot = tsb.tile([128, 512], F32, name="cvsb")
                    nc.any.tensor_copy(ot[:], pp[:])
                    nc.sync.dma_start(out[oc * 128:(oc + 1) * 128, s], ot[:])


def get_nc():
    if "nc" not in _CACHE:
        _CACHE["nc"] = _build()
    return _CACHE["nc"]


def make_in_maps(input_tensor1, input_tensor2, W1, b1, W2, b2, W3, b3, W4, b4):
    x1 = np.ascontiguousarray(np.asarray(input_tensor1, dtype=np.float32))
    x2 = np.ascontiguousarray(np.asarray(input_tensor2, dtype=np.float32))
    W1, W2, W3, W4 = (np.asarray(w, dtype=np.float32) for w in (W1, W2, W3, W4))
    b1, b2, b3, b4 = (np.asarray(b, dtype=np.float32) for b in (b1, b2, b3, b4))
    in_maps = []
    for p in range(8):
        b, g = p // 4, p % 4
        gs = slice(g * D, (g + 1) * D)
        in_maps.append({
            "x1": x1[b].reshape(C, N),
            "x2": x2[b].reshape(C, N),
            "w1t": np.ascontiguousarray(W1[gs, :].T),
            "w2t": np.ascontiguousarray(W2[gs, :].T),
            "w3t": np.ascontiguousarray(W3[gs, :].T),
            "w4gt": np.ascontiguousarray(W4[:, gs].T),
            "b1g": b1[gs].reshape(D, 1).copy(),
            "b2g": b2[gs].reshape(D, 1).copy(),
            "b3g": b3[gs].reshape(D, 1).copy(),
            "b4q": (b4 / 4.0).reshape(1, C).copy(),
        })
    return in_maps


def kernel(input_tensor1, input_tensor2, W1, b1, W2, b2, W3, b3, W4, b4):
    nc = get_nc()
    in_maps = make_in_maps(input_tensor1, input_tensor2,
                           W1, b1, W2, b2, W3, b3, W4, b4)
    res = run_bass_kernel_spmd(nc, in_maps, core_ids=list(range(8)))
    parts = [res.results[p]["out"] for p in range(8)]
    full = np.empty((2, C, 64, 64), dtype=np.float32)
    for b in range(2):
        acc = parts[b * 4] + parts[b * 4 + 1] + parts[b * 4 + 2] + parts[b * 4 + 3]
        # device layout is [o, w*64+h] -> [o, h, w]
        full[b] = acc.reshape(C, 64, 64).transpose(0, 2, 1)
    return full



# revision 9
# speedup vs baseline: 2.1684x; 2.1684x over previous
"""Trainium2 Bass kernel for nn_CrossAttention (dense_transformer).

Problem (per batch-head, 8 of them == 8 cores):
  K = conv1x1(x1, W1g, b1g)   -> [4096, 64]   (raw .view semantics)
  Q = conv1x1(x1, W2g, b2g)   -> [4096, 64]
  V = conv1x1(x2, W3g, b3g)   -> [4096, 64]
  A = K @ Q^T                  -> [4096, 4096]
  P = softmax(A, global over all 16.7M entries)
  O = P @ V                    -> [4096, 64]
  out_partial = (W4_gslice/S) @ O + b4/4   (summed over the 4 head-cores of a
                                            batch on the host during unshard)

Sharding: data-parallel over batch*head (core p <-> (b=p//4, g=p%4)).
Global softmax per bh row block needs no cross-device reduction.

Key implementation points:
 - softmax is computed as exp(A - 64) / sum(exp(A - 64)): a constant shift is
   mathematically identical to max-subtraction (softmax is shift invariant)
   and overflow-safe here (A values are ~N(0, 9.4^2), |A|max ~ 66 << 64+88).
 - the attention matrix is never materialized to HBM: A^T tiles are computed
   in PSUM, exp'd by the scalar engine into SBUF, and immediately consumed by
   the P@V matmul (flash-attention style, one pass).
 - row sums come for free from a ones-column appended to V.
 - all matmuls keep the contraction dim on partitions; K^T/Q^T layouts are
   produced with PE transposes of [128,64] tiles.
"""

import numpy as np

import concourse.bass as bass
import concourse.mybir as mybir
import concourse.tile as tile
from concourse import bacc
from concourse.bass_utils import run_bass_kernel_spmd
from concourse.masks import make_identity

F32 = mybir.dt.float32
F32R = mybir.dt.float32r
AF = mybir.ActivationFunctionType
AX = mybir.AxisListType


def _r(ap):
    # float32r matmul operands: 1 cycle/row on PE (vs 4 for fp32) when the
    # output free dim is >= 256; precision is ~22 mantissa bits, far inside
    # the 2e-2 tolerance.
    return ap.bitcast(F32R)

C = 256          # channels
N = 4096         # h*w
D = 64           # head dim (== w) and channels per head block
NJT = N // 128   # 32 j-tiles
NIB = N // 512   # 8 i-blocks
GRP = 3          # kq j-tiles per exp group (3 PSUM banks)
SHIFT = -64.0    # softmax stabilization shift (constant, shift-invariant)

_CACHE = {}


def _build():
    import os
    globals()['os'] = os
    build_stage = int(os.environ.get("KSTAGE", "3"))
    nc = bacc.Bacc("TRN2", target_bir_lowering=False)
    _emit(nc, build_stage)
    nc.finalize()
    return nc


def _emit(nc, build_stage):
    with tile.TileContext(nc) as tc, \
         tc.tile_pool(name="dram", bufs=1, space="DRAM") as dram, \
         tc.tile_pool(name="persist", bufs=1) as persist:
        # ---- I/O ----
        def ein(name, shape):
            return dram.tile(shape, F32, kind="ExternalInput", name=name,
                             uniquify=False)

        x1 = ein("x1", [C, N])
        x2 = ein("x2", [C, N])
        w1t = ein("w1t", [C, D])      # [c, cl] = W1[g-slice].T
        w2t = ein("w2t", [C, D])
        w3t = ein("w3t", [C, D])
        w4gt = ein("w4gt", [D, C])    # [cl, o] = W4[:, g-slice].T
        b1g = ein("b1g", [D, 1])
        b2g = ein("b2g", [D, 1])
        b3g = ein("b3g", [D, 1])
        b4q = ein("b4q", [1, C])      # b4 / 4
        out = dram.tile([C, N], F32, kind="ExternalOutput", name="out",
                        uniquify=False)

        # DRAM scratch (layout round-trips)
        knat_d = dram.tile([D, N], F32)   # [cl, (h w)] == [(cl h), w] flat
        qnat_d = dram.tile([D, N], F32)
        vnat_d = dram.tile([D, N], F32)
        onat_d = dram.tile([D, N], F32)   # [w, (cl h)]

        # ---- persistent SBUF ----
        ktsb = persist.tile([128, N], F32R)        # K^T [w, i] duplicated rows
        qtsb = persist.tile([128, N], F32R)        # Q^T [w, j] duplicated rows
        v2 = persist.tile([128, NJT * (D + 1)], F32R)   # V' tiles [j-loc, 65]
        otsb = persist.tile([D + 1, N], F32)      # O^T (+ rowsums in row 64)
        w4full = persist.tile([D + 1, C], F32)    # [cl(+bias row), o]
        w4s = persist.tile([D + 1, C], F32R)       # scaled by 1/S
        b1s = persist.tile([D, 1], F32)
        b2s = persist.tile([D, 1], F32)
        b3s = persist.tile([D, 1], F32)
        shift = persist.tile([128, 1], F32)
        ident = persist.tile([128, 128], F32)
        ones_r = persist.tile([1, D], F32)        # for 1/S broadcast matmul
        ssum = persist.tile([D + 1, NIB], F32)    # per-i-block sums (row 64)
        stot = persist.tile([D + 1, 1], F32)
        sinv = persist.tile([D + 1, 1], F32)
        sinv0 = persist.tile([1, 1], F32)
        sinv_b = persist.tile([D, 1], F32)

        nc.gpsimd.memset(shift[:], SHIFT)
        nc.gpsimd.memset(ones_r[:], 1.0)
        make_identity(nc, ident[:])
        nc.sync.dma_start(b1s[:], b1g[:])
        nc.sync.dma_start(b2s[:], b2g[:])
        nc.sync.dma_start(b3s[:], b3g[:])
        nc.sync.dma_start(w4full[0:D, :], w4gt[:])
        nc.sync.dma_start(w4full[D:D + 1, :], b4q[:])
        # ones column of V' (col 64 of each slot); memset can't write f32r,
        # so stage in f32 and round through a DVE copy
        v2v = v2.rearrange("p (j c) -> p j c", c=D + 1)
        ones_j = persist.tile([128, NJT], F32)
        nc.gpsimd.memset(ones_j[:], 1.0)
        nc.vector.tensor_copy(v2v[:, :, D], ones_j[:])

        with tc.tile_pool(name="xsb", bufs=1) as xpool, \
             tc.tile_pool(name="stage", bufs=4) as stage, \
             tc.tile_pool(name="sps", bufs=4, space="PSUM") as sps, \
             tc.tile_pool(name="tps", bufs=2, space="PSUM") as tps:
            # ---- load x1/x2 (chunked for DMA/compute overlap) ----
            x1a = xpool.tile([128, N], F32R)
            x1b = xpool.tile([128, N], F32R)
            x2a = xpool.tile([128, N], F32R)
            x2b = xpool.tile([128, N], F32R)
            for ch in range(8):
                s = slice(ch * 512, (ch + 1) * 512)
                nc.sync.dma_start(x1a[:, s], _r(x1[0:128, s]))
                nc.sync.dma_start(x1b[:, s], _r(x1[128:256, s]))
            for ch in range(8):
                s = slice(ch * 512, (ch + 1) * 512)
                nc.sync.dma_start(x2a[:, s], _r(x2[0:128, s]))
                nc.sync.dma_start(x2b[:, s], _r(x2[128:256, s]))

            # ---- projections: nat = W.T @ x + b  -> DRAM ----
            wsb = {}
            for nm, w in (("k", w1t), ("q", w2t), ("v", w3t)):
                wa = stage.tile([128, D], F32R, name=f"w{nm}a", bufs=1)
                wb = stage.tile([128, D], F32R, name=f"w{nm}b", bufs=1)
                nc.sync.dma_start(wa[:], _r(w[0:128, :]))
                nc.sync.dma_start(wb[:], _r(w[128:256, :]))
                wsb[nm] = (wa, wb)

            for nm, xa, xb, bia, dst in (("k", x1a, x1b, b1s, knat_d),
                                         ("q", x1a, x1b, b2s, qnat_d),
                                         ("v", x2a, x2b, b3s, vnat_d)):
                wa, wb = wsb[nm]
                for ch in range(8):
                    s = slice(ch * 512, (ch + 1) * 512)
                    pp = sps.tile([D, 512], F32, name="projps")
                    nc.tensor.matmul(pp[:], wa[:], xa[:, s],
                                     start=True, stop=False)
                    nc.tensor.matmul(pp[:], wb[:], xb[:, s],
                                     start=False, stop=True)
                    st = stage.tile([D, 512], F32, name="projsb")
                    nc.vector.tensor_scalar_add(st[:], pp[:], bia[:])
                    nc.sync.dma_start(dst[:, s], st[:])

            # ---- K^T / Q^T via PE transposes of [128, 64] tiles ----
            for src, dstt in ((knat_d, ktsb), (qnat_d, qtsb)):
                srcv = src.rearrange("a (h w) -> (a h) w", w=D)
                for t in range(NJT):
                    nat = stage.tile([128, D], F32, name="natt")
                    nc.sync.dma_start(nat[:], srcv[t * 128:(t + 1) * 128, :])
                    tp = tps.tile([D, 128], F32, name="tp")
                    nc.tensor.transpose(tp[:], nat[:], ident[:])
                    nc.any.tensor_copy(dstt[0:D, t * 128:(t + 1) * 128], tp[:])
                nc.sync.dma_start(dstt[D:128, :], dstt[0:D, :])

            # ---- V' load ----
            vv = vnat_d.rearrange("a (h w) -> (a h) w", w=D)
            for t in range(NJT):
                nc.sync.dma_start(v2v[:, t, 0:D], _r(vv[t * 128:(t + 1) * 128, :]))

        # ---- attention: one pass, ACT-bound ----
        if build_stage < 2:
            return
        groups = []
        jt0 = 0
        while jt0 < NJT:
            groups.append(list(range(jt0, min(jt0 + GRP, NJT))))
            jt0 += GRP

        with tc.tile_pool(name="kqps", bufs=2, space="PSUM") as kqps, \
             tc.tile_pool(name="ovps", bufs=2, space="PSUM") as ovps, \
             tc.tile_pool(name="ptsb", bufs=3) as ptsb:
            for ib in range(NIB):
                isl = slice(ib * 512, (ib + 1) * 512)
                ov = ovps.tile([D + 1, 512], F32, name="ov")
                for grp in groups:
                    ng = len(grp)
                    kq = kqps.tile([128, GRP * 512], F32, name="kq")
                    for gi, jt in enumerate(grp):
                        half = gi % 2
                        p0, p1 = half * D, half * D + D
                        nc.tensor.matmul(
                            kq[:, gi * 512:(gi + 1) * 512],
                            qtsb[p0:p1, jt * 128:(jt + 1) * 128],
                            ktsb[p0:p1, isl], start=True, stop=True,
                            tile_position=(p0, 0))
                    pt = ptsb.tile([128, GRP * 512], F32R, name="pt")
                    nc.scalar.activation(pt[:, 0:ng * 512], kq[:, 0:ng * 512],
                                         AF.Exp, bias=shift[:], scale=1.0)
                    for gi, jt in enumerate(grp):
                        nc.tensor.matmul(
                            ov[:], v2v[:, jt, :],
                            pt[:, gi * 512:(gi + 1) * 512],
                            start=(jt == 0), stop=(jt == NJT - 1))
                nc.vector.tensor_copy(otsb[:, isl], ov[:])
                nc.vector.reduce_sum(ssum[D:D + 1, ib:ib + 1],
                                     ov[D:D + 1, :], axis=AX.X)

        # ---- tail: S, scale W4, final conv ----
        if build_stage < 3:
            return
        ktail = int(os.environ.get("KTAIL", "9"))
        import os as _os  # noqa
        with tc.tile_pool(name="tailsb", bufs=4) as tsb, \
             tc.tile_pool(name="tailps", bufs=4, space="PSUM") as tps2:
            nc.vector.reduce_sum(stot[D:D + 1, :], ssum[D:D + 1, :], axis=AX.X)
            nc.vector.reciprocal(sinv[D:D + 1, :], stot[D:D + 1, :])
            if ktail < 2:
                return
            nc.sync.dma_start(sinv0[:], sinv[D:D + 1, :])
            if ktail < 3:
                return
            pb = tps2.tile([D, 1], F32, name="pb")
            nc.tensor.matmul(pb[:], ones_r[:], sinv0[:], start=True, stop=True)
            nc.vector.tensor_copy(sinv_b[:], pb[:])
            nc.vector.tensor_scalar_mul(w4s[0:D, :], w4full[0:D, :],
                                        sinv_b[:])
            nc.vector.tensor_copy(w4s[D:D + 1, :], w4full[D:D + 1, :])

            if ktail < 4:
                return
            nc.sync.dma_start(onat_d[:], otsb[0:D, :])
            oconv = tsb.tile([D + 1, N], F32R, bufs=1)
            ones_n = tsb.tile([1, N], F32, bufs=1)
            nc.gpsimd.memset(ones_n[:], 1.0)
            nc.vector.tensor_copy(oconv[D:D + 1, :], ones_n[:])
            # onat [w, (cl h)] -> oconv [cl, (w h)]
            if ktail < 5:
                return
            ov2 = onat_d.rearrange("w (cl h) -> cl w h", cl=D)
            oc3 = oconv[0:D, :].rearrange("cl (w h) -> cl w h", w=D)
            nc.sync.dma_start(oc3, _r(ov2))

            if ktail < 6:
                return
            for oc in range(2):
                for ch in range(8):
                    s = slice(ch * 512, (ch + 1) * 512)
                    pp = tps2.tile([128, 512], F32, name="cvps")
                    nc.tensor.matmul(pp[:], w4s[:, oc * 128:(oc + 1) * 128],
                                     oconv[:, s], start=True, stop=True)
                    ot = tsb.tile([128, 512], F32, name="cvsb")
                    nc.any.tensor_copy(ot[:], pp[:])
                    nc.sync.dma_start(out[oc * 128:(oc + 1) * 128, s], ot[:])


def get_nc():
    if "nc" not in _CACHE:
        _CACHE["nc"] = _build()
    return _CACHE["nc"]


def make_in_maps(input_tensor1, input_tensor2, W1, b1, W2, b2, W3, b3, W4, b4):
    x1 = np.ascontiguousarray(np.asarray(input_tensor1, dtype=np.float32))
    x2 = np.ascontiguousarray(np.asarray(input_tensor2, dtype=np.float32))
    W1, W2, W3, W4 = (np.asarray(w, dtype=np.float32) for w in (W1, W2, W3, W4))
    b1, b2, b3, b4 = (np.asarray(b, dtype=np.float32) for b in (b1, b2, b3, b4))
    in_maps = []
    for p in range(8):
        b, g = p // 4, p % 4
        gs = slice(g * D, (g + 1) * D)
        in_maps.append({
            "x1": x1[b].reshape(C, N),
            "x2": x2[b].reshape(C, N),
            "w1t": np.ascontiguousarray(W1[gs, :].T),
            "w2t": np.ascontiguousarray(W2[gs, :].T),
            "w3t": np.ascontiguousarray(W3[gs, :].T),
            "w4gt": np.ascontiguousarray(W4[:, gs].T),
            "b1g": b1[gs].reshape(D, 1).copy(),
            "b2g": b2[gs].reshape(D, 1).copy(),
            "b3g": b3[gs].reshape(D, 1).copy(),
            "b4q": (b4 / 4.0).reshape(1, C).copy(),
        })
    return in_maps


def kernel(input_tensor1, input_tensor2, W1, b1, W2, b2, W3, b3, W4, b4):
    nc = get_nc()
    in_maps = make_in_maps(input_tensor1, input_tensor2,
                           W1, b1, W2, b2, W3, b3, W4, b4)
    res = run_bass_kernel_spmd(nc, in_maps, core_ids=list(range(8)))
    parts = [res.results[p]["out"] for p in range(8)]
    full = np.empty((2, C, 64, 64), dtype=np.float32)
    for b in range(2):
        acc = parts[b * 4] + parts[b * 4 + 1] + parts[b * 4 + 2] + parts[b * 4 + 3]
        # device layout is [o, w*64+h] -> [o, h, w]
        full[b] = acc.reshape(C, 64, 64).transpose(0, 2, 1)
    return full

